# revision 2
# baseline (speedup 1.0000x reference)
"""BiMamba TRN2 Bass kernel v2, 8-core SPMD.

Shapes (hardcoded): B=1, L=2048, Dm=1024, Di=2048, N=16, K=4, R=64.
Tensor-parallel over d_inner (Di_loc=256 per core, 2 groups of 128).

v2: per-n scan tiles [128 ch, L].
 - dA = exp(A_n * dt) via Act per-partition scale (no broadcast matmuls)
 - depthwise conv as 4 diag-matmuls on PE (off DVE)
 - B/C broadcasts via DMA replication from AllReduce output
 - dBx / hc multiplies: DVE tensor_tensor or SDMA accum (CCE mult)
 - y = sum_n h_n*C_n via identity-matmul PSUM accumulation
 - bf16 collectives, AllReduce split per direction
"""
import sys

sys.path.insert(0, '/opt/trn_rl_repo')

import numpy as np
import ml_dtypes

import concourse.bacc as bacc
import concourse.mybir as mybir
from concourse.bass_utils import run_bass_kernel_spmd
from concourse.tile import TileContext

AF = mybir.ActivationFunctionType
ALU = mybir.AluOpType
F32 = mybir.dt.float32
BF16 = mybir.dt.bfloat16

L = 2048
DM = 1024
DI = 2048
N = 16
KC = 4
R = 64
NCORES = 8
DLOC = DI // NCORES      # 256
NG = DLOC // 128         # 2
CH = 512
TC4 = L // CH            # 4
PAD = KC - 1             # 3
XPW = L + 2 * PAD        # 2054

DBX_MODE = 'dve'         # 'dve' | 'dma'
HC_MODE = 'dve'          # 'dve' | 'dma'

_CACHE = {}


def _emit(nc, t):
    with TileContext(nc) as tc:
        with tc.tile_pool(name='sb', bufs=1) as P, \
             tc.tile_pool(name='ps', bufs=1, space='PSUM') as PS:

            # ---- h tiles + in_proj weights FIRST (head latency) ----
            hks = []
            for k in range(8):
                hk = P.tile([128, L], BF16, tag='bigh', bufs=8, name=f'hk{k}')
                nc.sync.dma_start(out=hk[:],
                                  in_=t['h_T'][k * 128:(k + 1) * 128, :])
                hks.append(hk)
            wks_g = []
            for g in range(NG):
                wks = []
                for m in range(8):
                    wk = P.tile([128, 128], BF16, tag='wstream', bufs=16,
                                name=f'wi{g}{m}')
                    nc.sync.dma_start(
                        out=wk[:],
                        in_=t['w_in_T'][m * 128:(m + 1) * 128,
                                        g * 128:(g + 1) * 128])
                    wks.append(wk)
                wks_g.append(wks)

            # ---- small persistent loads (gpsimd queue; needed later) ----
            def loadg(name, src, cols, dt=F32):
                out = []
                for g in range(NG):
                    tl = P.tile([128, cols], dt, tag=name, bufs=2,
                                name=f'{name}{g}')
                    nc.gpsimd.dma_start(out=tl[:],
                                        in_=src[g * 128:(g + 1) * 128, :])
                    out.append(tl)
                return out

            bdt_g = loadg('bdt', t['bdt'], 2)
            convb_g = loadg('convb', t['conv_b'], 2)
            d_g = loadg('dboth', t['d_both'], 2)
            a_g = loadg('acols', t['A_cols'], 2 * N)
            wx_g = loadg('wx', t['wx_T'], 2 * 96, BF16)

            ident = P.tile([128, 128], BF16, name='ident')
            nc.gpsimd.dma_start(out=ident[:], in_=t['ident'][:])
            wdt_t = P.tile([R, 2 * DLOC], BF16, name='wdt_t')
            nc.gpsimd.dma_start(out=wdt_t[:], in_=t['wdt_T'][:])
            wconv = {}
            for d in range(2):
                for g in range(NG):
                    for k in range(KC):
                        b = (d * NG + g) * KC + k
                        wc = P.tile([128, 128], BF16, tag='wconv', bufs=20,
                                    name=f'wc{d}{g}{k}')
                        nc.gpsimd.dma_start(
                            out=wc[:],
                            in_=t['w_conv'][b * 128:(b + 1) * 128, :])
                        wconv[(d, g, k)] = wc
            wdiag = {}
            for d in range(2):
                for g in range(NG):
                    b = 2 * NG * KC + d * NG + g
                    wc = P.tile([128, 128], BF16, tag='wconv', bufs=20,
                                name=f'wD{d}{g}')
                    nc.gpsimd.dma_start(
                        out=wc[:], in_=t['w_conv'][b * 128:(b + 1) * 128, :])
                    wdiag[(d, g)] = wc

            # ---- in_proj x -> x_pad (padded, shared by both dirs) ----
            x_pad = []
            for g in range(NG):
                wks = wks_g[g]
                xp = P.tile([128, XPW], BF16, tag='xpad', bufs=2,
                            name=f'xpad{g}')
                nc.gpsimd.memset(xp[:, :PAD], 0.0)
                nc.gpsimd.memset(xp[:, PAD + L:], 0.0)
                for c in range(TC4):
                    pp = PS.tile([128, CH], F32, tag='pa', bufs=2,
                                 name=f'px{g}{c}')
                    for m in range(8):
                        nc.tensor.matmul(
                            out=pp[:], lhsT=wks[m][:],
                            rhs=hks[m][:, c * CH:(c + 1) * CH],
                            start=(m == 0), stop=(m == 7))
                    nc.scalar.activation(
                        out=xp[:, PAD + c * CH:PAD + (c + 1) * CH],
                        in_=pp[:], func=AF.Copy)
                x_pad.append(xp)

            # ---- conv (diag matmuls) + silu -> xa[(d,g)] ----
            xa = {}

            def emit_conv(d):
                for g in range(NG):
                    xat = P.tile([128, L], BF16, tag='xa', bufs=4,
                                 name=f'xa{d}{g}')
                    for c in range(TC4):
                        pc = PS.tile([128, CH], F32, tag='pa', bufs=2,
                                     name=f'pc{d}{g}{c}')
                        for k in range(KC):
                            off = k if d == 0 else PAD + k
                            nc.tensor.matmul(
                                out=pc[:], lhsT=wconv[(d, g, k)][:],
                                rhs=x_pad[g][:, off + c * CH:off + c * CH + CH],
                                start=(k == 0), stop=(k == KC - 1))
                        nc.scalar.activation(
                            out=xat[:, c * CH:(c + 1) * CH], in_=pc[:],
                            func=AF.Silu, bias=convb_g[g][:, d:d + 1])
                    xa[(d, g)] = xat

            def emit_wx(d):
                xdbl = P.tile([96, L], BF16, tag='xdbl', bufs=2,
                              name=f'xdbl{d}')
                for c in range(TC4):
                    pw = PS.tile([96, CH], F32, tag='pb', bufs=2,
                                 name=f'pw{d}{c}')
                    for g in range(NG):
                        nc.tensor.matmul(
                            out=pw[:], lhsT=wx_g[g][:, 96 * d:96 * (d + 1)],
                            rhs=xa[(d, g)][:, c * CH:(c + 1) * CH],
                            start=(g == 0), stop=(g == NG - 1))
                    nc.scalar.activation(out=xdbl[:, c * CH:(c + 1) * CH],
                                         in_=pw[:], func=AF.Copy)
                nc.sync.dma_start(out=t[f'ar_in{d}'][:], in_=xdbl[:])
                nc.gpsimd.collective_compute(
                    'AllReduce', ALU.add,
                    replica_groups=[list(range(NCORES))],
                    ins=[t[f'ar_in{d}'][:]], outs=[t[f'ar_out{d}'][:]])

            emit_conv(0)
            emit_wx(0)
            emit_conv(1)

            # ---- z -> silu(z) (overlaps AR_fwd) ----
            z_t = []
            for g in range(NG):
                wzs = []
                for m in range(8):
                    wz = P.tile([128, 128], BF16, tag='wstream', bufs=16,
                                name=f'wz{g}{m}')
                    nc.sync.dma_start(
                        out=wz[:],
                        in_=t['w_in_T'][m * 128:(m + 1) * 128,
                                        DLOC + g * 128:DLOC + (g + 1) * 128])
                    wzs.append(wz)
                sgt = P.tile([128, L], BF16, tag='sg', bufs=2, name=f'sg{g}')
                for c in range(TC4):
                    pz = PS.tile([128, CH], F32, tag='pa', bufs=2,
                                 name=f'pz{g}{c}')
                    for m in range(8):
                        nc.tensor.matmul(
                            out=pz[:], lhsT=wzs[m][:],
                            rhs=hks[m][:, c * CH:(c + 1) * CH],
                            start=(m == 0), stop=(m == 7))
                    nc.scalar.activation(out=sgt[:, c * CH:(c + 1) * CH],
                                         in_=pz[:], func=AF.Silu)
                z_t.append(sgt)

            emit_wx(1)

            # ---- post-AR per direction: dt (softplus) + dtx ----
            dt_t = {}
            dtx_t = {}

            drs = {}

            def emit_dt(d, g):
                if d not in drs:
                    dr = P.tile([R, L], BF16, tag='dr', bufs=2, name=f'dr{d}')
                    nc.sync.dma_start(out=dr[:], in_=t[f'ar_out{d}'][0:R, :])
                    drs[d] = dr
                dr = drs[d]
                et = P.tile([128, L], F32, tag='bigf', bufs=4,
                            name=f'et{d}{g}')
                for c in range(TC4):
                    pd = PS.tile([128, CH], F32, tag='pb', bufs=2,
                                 name=f'pd{d}{g}{c}')
                    nc.tensor.matmul(
                        out=pd[:],
                        lhsT=wdt_t[:, d * DLOC + g * 128:
                                   d * DLOC + (g + 1) * 128],
                        rhs=dr[:, c * CH:(c + 1) * CH],
                        start=True, stop=True)
                    nc.scalar.activation(out=et[:, c * CH:(c + 1) * CH],
                                         in_=pd[:], func=AF.Exp,
                                         bias=bdt_g[g][:, d:d + 1])
                dtt = P.tile([128, L], BF16, tag='dt', bufs=4,
                             name=f'dt{d}{g}')
                nc.scalar.activation(out=dtt[:], in_=et[:], func=AF.Ln,
                                     bias=1.0)
                dt_t[(d, g)] = dtt
                dtx = P.tile([128, L], BF16, tag='dtx', bufs=4,
                             name=f'dtx{d}{g}')
                nc.vector.tensor_tensor(out=dtx[:], in0=dtt[:],
                                        in1=xa[(d, g)][:], op=ALU.mult)
                dtx_t[(d, g)] = dtx

            emit_dt(0, 0)

            # ---- scan loop (g outer: py reuses one 4-bank PSUM slot) ----
            yg_t = []
            for g in range(NG):
                py = PS.tile([128, L], F32, tag='py', bufs=1, name=f'py{g}')
                # D-residual opens the accumulation (depends only on xa)
                for dd in range(2):
                    for c in range(TC4):
                        nc.tensor.matmul(
                            out=py[:, c * CH:(c + 1) * CH],
                            lhsT=wdiag[(dd, g)][:],
                            rhs=xa[(dd, g)][:, c * CH:(c + 1) * CH],
                            start=(dd == 0), stop=False)
                for d in range(2):
                    if (d, g) not in dt_t:
                        emit_dt(d, g)
                    for n in range(N):
                        if n == 8:
                            nxt = (1, g) if d == 0 else (0, g + 1)
                            if nxt[1] < NG and nxt not in dt_t:
                                emit_dt(*nxt)
                        brow = t[f'ar_out{d}'][R + n:R + n + 1, :]
                        crow = t[f'ar_out{d}'][R + N + n:R + N + n + 1, :]
                        if DBX_MODE == 'dve':
                            bB = P.tile([128, L], BF16, tag='bc', bufs=8,
                                        name=f'bB{g}{d}{n}')
                            nc.sync.dma_start(
                                out=bB[:], in_=brow.broadcast_to([128, L]))
                        if HC_MODE == 'dve':
                            bC = P.tile([128, L], BF16, tag='bc', bufs=8,
                                        name=f'bC{g}{d}{n}')
                            nc.scalar.dma_start(
                                out=bC[:], in_=crow.broadcast_to([128, L]))
                        dA = P.tile([128, L], F32, tag='bigf', bufs=4,
                                    name=f'dA{g}{d}{n}')
                        nc.scalar.activation(
                            out=dA[:], in_=dt_t[(d, g)][:], func=AF.Exp,
                            scale=a_g[g][:, d * N + n:d * N + n + 1])
                        dBx = P.tile([128, L], BF16, tag='bigh', bufs=8,
                                     name=f'dBx{g}{d}{n}')
                        if DBX_MODE == 'dma':
                            nc.sync.dma_start(out=dBx[:],
                                              in_=brow.broadcast_to([128, L]))
                            nc.gpsimd.dma_start(out=dBx[:],
                                                in_=dtx_t[(d, g)][:],
                                                accum_op=ALU.mult)
                        else:
                            nc.vector.tensor_tensor(
                                out=dBx[:], in0=dtx_t[(d, g)][:], in1=bB[:],
                                op=ALU.mult)
                        h_t = P.tile([128, L], BF16, tag='bigh', bufs=8,
                                     name=f'h{g}{d}{n}')
                        if d == 0:
                            nc.vector.tensor_tensor_scan(
                                out=h_t[:], data0=dA[:], data1=dBx[:],
                                initial=0.0, op0=ALU.mult, op1=ALU.add)
                        else:
                            nc.vector.tensor_tensor_scan(
                                out=h_t[:, ::-1], data0=dA[:, ::-1],
                                data1=dBx[:, ::-1],
                                initial=0.0, op0=ALU.mult, op1=ALU.add)
                        hc = P.tile([128, L], BF16, tag='bigh', bufs=8,
                                    name=f'hc{g}{d}{n}')
                        if HC_MODE == 'dma':
                            nc.sync.dma_start(out=hc[:],
                                              in_=crow.broadcast_to([128, L]))
                            nc.gpsimd.dma_start(out=hc[:], in_=h_t[:],
                                                accum_op=ALU.mult)
                        else:
                            nc.vector.tensor_tensor(
                                out=hc[:], in0=h_t[:], in1=bC[:],
                                op=ALU.mult)
                        for c in range(TC4):
                            nc.tensor.matmul(
                                out=py[:, c * CH:(c + 1) * CH],
                                lhsT=ident[:],
                                rhs=hc[:, c * CH:(c + 1) * CH],
                                start=False,
                                stop=(d == 1 and n == N - 1))

                # gating for this group (releases py banks)
                yg = P.tile([128, L], BF16, tag='yg', bufs=2, name=f'yg{g}')
                nc.vector.tensor_tensor(out=yg[:], in0=py[:], in1=z_t[g][:],
                                        op=ALU.mult)
                yg_t.append(yg)

                # out_proj partial for this g; g=0 overlaps g=1's scan loop
                for mb in range(8):
                    wo = P.tile([128, 128], BF16, tag='wstream', bufs=16,
                                name=f'wo{mb}{g}')
                    nc.sync.dma_start(
                        out=wo[:],
                        in_=t['w_out_T'][g * 128:(g + 1) * 128,
                                         mb * 128:(mb + 1) * 128])
                    ost = P.tile([128, L], BF16, tag='ost', bufs=3,
                                 name=f'os{mb}{g}')
                    if g == 0:
                        po = PS.tile([128, L], F32, tag='py', bufs=1,
                                     name=f'po{mb}{g}')
                        for c in range(TC4):
                            nc.tensor.matmul(
                                out=po[:, c * CH:(c + 1) * CH], lhsT=wo[:],
                                rhs=yg[:, c * CH:(c + 1) * CH],
                                start=True, stop=True)
                        nc.scalar.activation(out=ost[:], in_=po[:],
                                             func=AF.Copy)
                    else:
                        for c in range(TC4):
                            po = PS.tile([128, CH], F32, tag='pa', bufs=2,
                                         name=f'po{mb}{g}{c}')
                            nc.tensor.matmul(
                                out=po[:], lhsT=wo[:],
                                rhs=yg[:, c * CH:(c + 1) * CH],
                                start=True, stop=True)
                            if (mb * TC4 + c) % 2 == 0:
                                nc.scalar.activation(
                                    out=ost[:, c * CH:(c + 1) * CH],
                                    in_=po[:], func=AF.Copy)
                            else:
                                nc.vector.tensor_copy(
                                    ost[:, c * CH:(c + 1) * CH], po[:])
                    nc.sync.dma_start(
                        out=t[f'rs_in{g}'][mb * 128:(mb + 1) * 128, :],
                        in_=ost[:])
                # RS for this g; g=0's runs hidden under g=1's scan loop
                nc.gpsimd.collective_compute(
                    'ReduceScatter', ALU.add,
                    replica_groups=[list(range(NCORES))],
                    ins=[t[f'rs_in{g}'][:]], outs=[t[f'rs_out{g}'][:]])


            ro0 = P.tile([128, L], BF16, tag='yg', bufs=2, name='ro0')
            nc.sync.dma_start(out=ro0[:], in_=t['rs_out0'][:])
            ro1 = P.tile([128, L], BF16, tag='ost', bufs=3, name='ro1')
            nc.sync.dma_start(out=ro1[:], in_=t['rs_out1'][:])
            outf = P.tile([128, L], BF16, tag='ost', bufs=3, name='outf')
            nc.vector.tensor_tensor(out=outf[:], in0=ro0[:], in1=ro1[:],
                                    op=ALU.add)
            nc.sync.dma_start(out=t['out_mT'][:], in_=outf[:])


def _build():
    nc = bacc.Bacc(None, target_bir_lowering=False)

    def inp(name, shape, dt=F32):
        return nc.declare_dram_parameter(name, shape, dt, isOutput=False)

    t = {
        'h_T': inp('h_T', [DM, L], BF16),
        'w_in_T': inp('w_in_T', [DM, 2 * DLOC], BF16),
        'w_conv': inp('w_conv', [(2 * NG * KC + 2 * NG) * 128, 128], BF16),
        'wx_T': inp('wx_T', [DLOC, 2 * 96], BF16),
        'wdt_T': inp('wdt_T', [R, 2 * DLOC], BF16),
        'w_out_T': inp('w_out_T', [DLOC, DM], BF16),
        'bdt': inp('bdt', [DLOC, 2]),
        'conv_b': inp('conv_b', [DLOC, 2]),
        'd_both': inp('d_both', [DLOC, 2]),
        'A_cols': inp('A_cols', [DLOC, 2 * N]),
        'ident': inp('ident', [128, 128], BF16),
        'out_mT': nc.declare_dram_parameter('out_mT', [128, L], BF16,
                                            isOutput=True),
        'rs_in0': nc.dram_tensor('rs_in0', [DM, L], BF16),
        'rs_in1': nc.dram_tensor('rs_in1', [DM, L], BF16),
        'rs_out0': nc.dram_tensor('rs_out0', [128, L], BF16),
        'rs_out1': nc.dram_tensor('rs_out1', [128, L], BF16),
    }
    for d in range(2):
        t[f'ar_in{d}'] = nc.dram_tensor(f'ar_in{d}', [96, L], BF16)
        t[f'ar_out{d}'] = nc.dram_tensor(f'ar_out{d}', [96, L], BF16,
                                         addr_space='Shared')
    _emit(nc, t)
    nc.compile()
    return nc


def prepare_in_maps(inputs):
    f32 = np.float32
    bf16 = ml_dtypes.bfloat16
    h = np.asarray(inputs['hidden_states'], f32)[0]        # [L, DM]
    h_T = np.ascontiguousarray(h.T).astype(bf16)
    W_in = np.asarray(inputs['W_in'], f32)
    W_out = np.asarray(inputs['W_out'], f32)

    maps = []
    for c in range(NCORES):
        sl = slice(c * DLOC, (c + 1) * DLOC)
        cw_f = np.asarray(inputs['conv_w_fwd'], f32)[sl]   # [DLOC, 4]
        cw_r = np.asarray(inputs['conv_w_rev'], f32)[sl]
        w_conv = np.zeros(((2 * NG * KC + 2 * NG) * 128, 128), f32)
        for d in range(2):
            for g in range(NG):
                for k in range(KC):
                    b = (d * NG + g) * KC + k
                    ch = slice(g * 128, (g + 1) * 128)
                    tap = cw_f[ch, k] if d == 0 else cw_r[ch, KC - 1 - k]
                    w_conv[b * 128:(b + 1) * 128, :] = np.diag(tap)
        dp_f = np.asarray(inputs['D_fwd'], f32)[sl]
        dp_r = np.asarray(inputs['D_rev'], f32)[sl]
        for d in range(2):
            for g in range(NG):
                b = 2 * NG * KC + d * NG + g
                ch = slice(g * 128, (g + 1) * 128)
                w_conv[b * 128:(b + 1) * 128, :] = np.diag(
                    (dp_f if d == 0 else dp_r)[ch])
        a_cols = np.concatenate(
            [-np.exp(np.asarray(inputs['A_log_fwd'], f32)[sl]),
             -np.exp(np.asarray(inputs['A_log_rev'], f32)[sl])], axis=1)
        w_in_T = np.concatenate(
            [W_in[sl].T, W_in[DI + c * DLOC:DI + (c + 1) * DLOC].T], axis=1)
        m = {
            'h_T': h_T,
            'w_in_T': np.ascontiguousarray(w_in_T).astype(bf16),
            'w_conv': w_conv.astype(bf16),
            'wx_T': np.ascontiguousarray(np.concatenate(
                [np.asarray(inputs['Wx_fwd'], f32)[:, sl].T,
                 np.asarray(inputs['Wx_rev'], f32)[:, sl].T],
                axis=1)).astype(bf16),
            'wdt_T': np.ascontiguousarray(np.concatenate(
                [np.asarray(inputs['Wdt_fwd'], f32)[sl].T,
                 np.asarray(inputs['Wdt_rev'], f32)[sl].T],
                axis=1)).astype(bf16),
            'w_out_T': np.ascontiguousarray(W_out[:, sl].T).astype(bf16),
            'bdt': np.ascontiguousarray(np.stack(
                [np.asarray(inputs['bdt_fwd'], f32)[sl],
                 np.asarray(inputs['bdt_rev'], f32)[sl]], axis=1)),
            'conv_b': np.ascontiguousarray(np.stack(
                [np.asarray(inputs['conv_b_fwd'], f32)[sl],
                 np.asarray(inputs['conv_b_rev'], f32)[sl]], axis=1)),
            'd_both': np.ascontiguousarray(np.stack(
                [np.asarray(inputs['D_fwd'], f32)[sl],
                 np.asarray(inputs['D_rev'], f32)[sl]], axis=1)),
            'A_cols': np.ascontiguousarray(a_cols),
            'ident': np.eye(128, dtype=f32).astype(bf16),
        }
        maps.append(m)
    return maps


def get_nc():
    if 'nc' not in _CACHE:
        _CACHE['nc'] = _build()
    return _CACHE['nc']


def run(inputs, **kw):
    nc = get_nc()
    maps = prepare_in_maps(inputs)
    res = run_bass_kernel_spmd(nc, maps, list(range(NCORES)), **kw)
    out_T = np.concatenate(
        [np.asarray(res.results[c]['out_mT'], np.float32)
         for c in range(NCORES)], axis=0)                 # [DM, L]
    out = np.ascontiguousarray(out_T.T)[None]             # [1, L, DM]
    return out.astype(np.float32), res


def kernel(**inputs):
    out, _ = run(inputs)
    return out


# revision 3
# speedup vs baseline: 1.0543x; 1.0543x over previous
"""BiMamba TRN2 Bass kernel v2, 8-core SPMD.

Shapes (hardcoded): B=1, L=2048, Dm=1024, Di=2048, N=16, K=4, R=64.
Tensor-parallel over d_inner (Di_loc=256 per core, 2 groups of 128).

v2: per-n scan tiles [128 ch, L].
 - dA = exp(A_n * dt) via Act per-partition scale (no broadcast matmuls)
 - depthwise conv as 4 diag-matmuls on PE (off DVE)
 - B/C broadcasts via DMA replication from AllReduce output
 - dBx / hc multiplies: DVE tensor_tensor or SDMA accum (CCE mult)
 - y = sum_n h_n*C_n via identity-matmul PSUM accumulation
 - bf16 collectives, AllReduce split per direction
"""
import sys

sys.path.insert(0, '/opt/trn_rl_repo')

import numpy as np
import ml_dtypes

import concourse.bacc as bacc
import concourse.mybir as mybir
from concourse.bass_utils import run_bass_kernel_spmd
from concourse.tile import TileContext

AF = mybir.ActivationFunctionType
ALU = mybir.AluOpType
F32 = mybir.dt.float32
BF16 = mybir.dt.bfloat16

L = 2048
DM = 1024
DI = 2048
N = 16
KC = 4
R = 64
NCORES = 8
DLOC = DI // NCORES      # 256
NG = DLOC // 128         # 2
CH = 512
TC4 = L // CH            # 4
PAD = KC - 1             # 3
XPW = L + 2 * PAD        # 2054

DBX_MODE = 'dve'         # 'dve' | 'dma'
HC_MODE = 'dve'          # 'dve' | 'dma'

_CACHE = {}


def _emit(nc, t):
    with TileContext(nc) as tc:
        with tc.tile_pool(name='sb', bufs=1) as P, \
             tc.tile_pool(name='ps', bufs=1, space='PSUM') as PS:

            # ---- warm-up collective: absorb cc-stream setup latency ----
            wtile = P.tile([8, 16], BF16, name='wtile')
            nc.gpsimd.memset(wtile[:], 0.0)
            nc.gpsimd.dma_start(out=t['warm_in'][:], in_=wtile[:])
            nc.gpsimd.collective_compute(
                'AllReduce', ALU.add, replica_groups=[list(range(NCORES))],
                ins=[t['warm_in'][:]], outs=[t['warm_out'][:]])

            # ---- h tiles + in_proj weights FIRST (head latency) ----
            hks = []
            for k in range(8):
                hk = P.tile([128, L], BF16, tag='bigh', bufs=8, name=f'hk{k}')
                nc.sync.dma_start(out=hk[:],
                                  in_=t['h_T'][k * 128:(k + 1) * 128, :])
                hks.append(hk)
            wks_g = []
            for g in range(NG):
                wks = []
                for m in range(8):
                    wk = P.tile([128, 128], BF16, tag='wstream', bufs=16,
                                name=f'wi{g}{m}')
                    nc.sync.dma_start(
                        out=wk[:],
                        in_=t['w_in_T'][m * 128:(m + 1) * 128,
                                        g * 128:(g + 1) * 128])
                    wks.append(wk)
                wks_g.append(wks)

            # ---- small persistent loads (gpsimd queue; needed later) ----
            def loadg(name, src, cols, dt=F32):
                out = []
                for g in range(NG):
                    tl = P.tile([128, cols], dt, tag=name, bufs=2,
                                name=f'{name}{g}')
                    nc.gpsimd.dma_start(out=tl[:],
                                        in_=src[g * 128:(g + 1) * 128, :])
                    out.append(tl)
                return out

            bdt_g = loadg('bdt', t['bdt'], 2)
            convb_g = loadg('convb', t['conv_b'], 2)
            d_g = loadg('dboth', t['d_both'], 2)
            a_g = loadg('acols', t['A_cols'], 2 * N)
            wx_g = loadg('wx', t['wx_T'], 2 * 96, BF16)

            ident = P.tile([128, 128], BF16, name='ident')
            nc.gpsimd.dma_start(out=ident[:], in_=t['ident'][:])
            wdt_t = P.tile([R, 2 * DLOC], BF16, name='wdt_t')
            nc.gpsimd.dma_start(out=wdt_t[:], in_=t['wdt_T'][:])
            wconv = {}
            for d in range(2):
                for g in range(NG):
                    for k in range(KC):
                        b = (d * NG + g) * KC + k
                        wc = P.tile([128, 128], BF16, tag='wconv', bufs=20,
                                    name=f'wc{d}{g}{k}')
                        nc.gpsimd.dma_start(
                            out=wc[:],
                            in_=t['w_conv'][b * 128:(b + 1) * 128, :])
                        wconv[(d, g, k)] = wc
            wdiag = {}
            for d in range(2):
                for g in range(NG):
                    b = 2 * NG * KC + d * NG + g
                    wc = P.tile([128, 128], BF16, tag='wconv', bufs=20,
                                name=f'wD{d}{g}')
                    nc.gpsimd.dma_start(
                        out=wc[:], in_=t['w_conv'][b * 128:(b + 1) * 128, :])
                    wdiag[(d, g)] = wc

            # ---- in_proj x -> x_pad (padded, shared by both dirs) ----
            x_pad = []
            for g in range(NG):
                wks = wks_g[g]
                xp = P.tile([128, XPW], BF16, tag='xpad', bufs=2,
                            name=f'xpad{g}')
                nc.gpsimd.memset(xp[:, :PAD], 0.0)
                nc.gpsimd.memset(xp[:, PAD + L:], 0.0)
                for c in range(TC4):
                    pp = PS.tile([128, CH], F32, tag='pa', bufs=2,
                                 name=f'px{g}{c}')
                    for m in range(8):
                        nc.tensor.matmul(
                            out=pp[:], lhsT=wks[m][:],
                            rhs=hks[m][:, c * CH:(c + 1) * CH],
                            start=(m == 0), stop=(m == 7))
                    nc.scalar.activation(
                        out=xp[:, PAD + c * CH:PAD + (c + 1) * CH],
                        in_=pp[:], func=AF.Copy)
                x_pad.append(xp)

            # ---- conv (diag matmuls) + silu -> xa[(d,g)] ----
            xa = {}

            def emit_conv(d):
                for g in range(NG):
                    xat = P.tile([128, L], BF16, tag='xa', bufs=4,
                                 name=f'xa{d}{g}')
                    for c in range(TC4):
                        pc = PS.tile([128, CH], F32, tag='pa', bufs=2,
                                     name=f'pc{d}{g}{c}')
                        for k in range(KC):
                            off = k if d == 0 else PAD + k
                            nc.tensor.matmul(
                                out=pc[:], lhsT=wconv[(d, g, k)][:],
                                rhs=x_pad[g][:, off + c * CH:off + c * CH + CH],
                                start=(k == 0), stop=(k == KC - 1))
                        nc.scalar.activation(
                            out=xat[:, c * CH:(c + 1) * CH], in_=pc[:],
                            func=AF.Silu, bias=convb_g[g][:, d:d + 1])
                    xa[(d, g)] = xat

            def emit_wx(d):
                xdbl = P.tile([96, L], BF16, tag='xdbl', bufs=2,
                              name=f'xdbl{d}')
                for c in range(TC4):
                    pw = PS.tile([96, CH], F32, tag='pb', bufs=2,
                                 name=f'pw{d}{c}')
                    for g in range(NG):
                        nc.tensor.matmul(
                            out=pw[:], lhsT=wx_g[g][:, 96 * d:96 * (d + 1)],
                            rhs=xa[(d, g)][:, c * CH:(c + 1) * CH],
                            start=(g == 0), stop=(g == NG - 1))
                    nc.scalar.activation(out=xdbl[:, c * CH:(c + 1) * CH],
                                         in_=pw[:], func=AF.Copy)
                nc.sync.dma_start(out=t[f'ar_in{d}'][:], in_=xdbl[:])
                nc.gpsimd.collective_compute(
                    'AllReduce', ALU.add,
                    replica_groups=[list(range(NCORES))],
                    ins=[t[f'ar_in{d}'][:]], outs=[t[f'ar_out{d}'][:]])

            emit_conv(0)
            emit_wx(0)
            emit_conv(1)

            # ---- z -> silu(z) (overlaps AR_fwd) ----
            z_t = []
            for g in range(NG):
                wzs = []
                for m in range(8):
                    wz = P.tile([128, 128], BF16, tag='wstream', bufs=16,
                                name=f'wz{g}{m}')
                    nc.sync.dma_start(
                        out=wz[:],
                        in_=t['w_in_T'][m * 128:(m + 1) * 128,
                                        DLOC + g * 128:DLOC + (g + 1) * 128])
                    wzs.append(wz)
                sgt = P.tile([128, L], BF16, tag='sg', bufs=2, name=f'sg{g}')
                for c in range(TC4):
                    pz = PS.tile([128, CH], F32, tag='pa', bufs=2,
                                 name=f'pz{g}{c}')
                    for m in range(8):
                        nc.tensor.matmul(
                            out=pz[:], lhsT=wzs[m][:],
                            rhs=hks[m][:, c * CH:(c + 1) * CH],
                            start=(m == 0), stop=(m == 7))
                    nc.scalar.activation(out=sgt[:, c * CH:(c + 1) * CH],
                                         in_=pz[:], func=AF.Silu)
                z_t.append(sgt)

            emit_wx(1)

            # ---- post-AR per direction: dt (softplus) + dtx ----
            dt_t = {}
            dtx_t = {}

            drs = {}

            def emit_dt(d, g):
                if d not in drs:
                    dr = P.tile([R, L], BF16, tag='dr', bufs=2, name=f'dr{d}')
                    nc.sync.dma_start(out=dr[:], in_=t[f'ar_out{d}'][0:R, :])
                    drs[d] = dr
                dr = drs[d]
                et = P.tile([128, L], F32, tag='bigf', bufs=4,
                            name=f'et{d}{g}')
                for c in range(TC4):
                    pd = PS.tile([128, CH], F32, tag='pb', bufs=2,
                                 name=f'pd{d}{g}{c}')
                    nc.tensor.matmul(
                        out=pd[:],
                        lhsT=wdt_t[:, d * DLOC + g * 128:
                                   d * DLOC + (g + 1) * 128],
                        rhs=dr[:, c * CH:(c + 1) * CH],
                        start=True, stop=True)
                    nc.scalar.activation(out=et[:, c * CH:(c + 1) * CH],
                                         in_=pd[:], func=AF.Exp,
                                         bias=bdt_g[g][:, d:d + 1])
                dtt = P.tile([128, L], BF16, tag='dt', bufs=4,
                             name=f'dt{d}{g}')
                nc.scalar.activation(out=dtt[:], in_=et[:], func=AF.Ln,
                                     bias=1.0)
                dt_t[(d, g)] = dtt
                dtx = P.tile([128, L], BF16, tag='dtx', bufs=4,
                             name=f'dtx{d}{g}')
                nc.vector.tensor_tensor(out=dtx[:], in0=dtt[:],
                                        in1=xa[(d, g)][:], op=ALU.mult)
                dtx_t[(d, g)] = dtx

            emit_dt(0, 0)

            # ---- scan loop (g outer: py reuses one 4-bank PSUM slot) ----
            yg_t = []
            for g in range(NG):
                py = PS.tile([128, L], F32, tag='py', bufs=1, name=f'py{g}')
                # D-residual opens the accumulation (depends only on xa)
                for dd in range(2):
                    for c in range(TC4):
                        nc.tensor.matmul(
                            out=py[:, c * CH:(c + 1) * CH],
                            lhsT=wdiag[(dd, g)][:],
                            rhs=xa[(dd, g)][:, c * CH:(c + 1) * CH],
                            start=(dd == 0), stop=False)
                for d in range(2):
                    if (d, g) not in dt_t:
                        emit_dt(d, g)
                    for n in range(N):
                        if n == 8:
                            nxt = (1, g) if d == 0 else (0, g + 1)
                            if nxt[1] < NG and nxt not in dt_t:
                                emit_dt(*nxt)
                        brow = t[f'ar_out{d}'][R + n:R + n + 1, :]
                        crow = t[f'ar_out{d}'][R + N + n:R + N + n + 1, :]
                        if DBX_MODE == 'dve':
                            bB = P.tile([128, L], BF16, tag='bc', bufs=8,
                                        name=f'bB{g}{d}{n}')
                            nc.sync.dma_start(
                                out=bB[:], in_=brow.broadcast_to([128, L]))
                        if HC_MODE == 'dve':
                            bC = P.tile([128, L], BF16, tag='bc', bufs=8,
                                        name=f'bC{g}{d}{n}')
                            nc.scalar.dma_start(
                                out=bC[:], in_=crow.broadcast_to([128, L]))
                        dA = P.tile([128, L], F32, tag='bigf', bufs=4,
                                    name=f'dA{g}{d}{n}')
                        nc.scalar.activation(
                            out=dA[:], in_=dt_t[(d, g)][:], func=AF.Exp,
                            scale=a_g[g][:, d * N + n:d * N + n + 1])
                        dBx = P.tile([128, L], BF16, tag='bigh', bufs=8,
                                     name=f'dBx{g}{d}{n}')
                        if DBX_MODE == 'dma':
                            nc.sync.dma_start(out=dBx[:],
                                              in_=brow.broadcast_to([128, L]))
                            nc.gpsimd.dma_start(out=dBx[:],
                                                in_=dtx_t[(d, g)][:],
                                                accum_op=ALU.mult)
                        else:
                            nc.vector.tensor_tensor(
                                out=dBx[:], in0=dtx_t[(d, g)][:], in1=bB[:],
                                op=ALU.mult)
                        h_t = P.tile([128, L], BF16, tag='bigh', bufs=8,
                                     name=f'h{g}{d}{n}')
                        if d == 0:
                            nc.vector.tensor_tensor_scan(
                                out=h_t[:], data0=dA[:], data1=dBx[:],
                                initial=0.0, op0=ALU.mult, op1=ALU.add)
                        else:
                            nc.vector.tensor_tensor_scan(
                                out=h_t[:, ::-1], data0=dA[:, ::-1],
                                data1=dBx[:, ::-1],
                                initial=0.0, op0=ALU.mult, op1=ALU.add)
                        hc = P.tile([128, L], BF16, tag='bigh', bufs=8,
                                    name=f'hc{g}{d}{n}')
                        if HC_MODE == 'dma':
                            nc.sync.dma_start(out=hc[:],
                                              in_=crow.broadcast_to([128, L]))
                            nc.gpsimd.dma_start(out=hc[:], in_=h_t[:],
                                                accum_op=ALU.mult)
                        else:
                            nc.vector.tensor_tensor(
                                out=hc[:], in0=h_t[:], in1=bC[:],
                                op=ALU.mult)
                        for c in range(TC4):
                            nc.tensor.matmul(
                                out=py[:, c * CH:(c + 1) * CH],
                                lhsT=ident[:],
                                rhs=hc[:, c * CH:(c + 1) * CH],
                                start=False,
                                stop=(d == 1 and n == N - 1))

                # gating for this group (releases py banks)
                yg = P.tile([128, L], BF16, tag='yg', bufs=2, name=f'yg{g}')
                nc.vector.tensor_tensor(out=yg[:], in0=py[:], in1=z_t[g][:],
                                        op=ALU.mult)
                yg_t.append(yg)

                # out_proj partial for this g; g=0 overlaps g=1's scan loop
                for mb in range(8):
                    wo = P.tile([128, 128], BF16, tag='wstream', bufs=16,
                                name=f'wo{mb}{g}')
                    nc.sync.dma_start(
                        out=wo[:],
                        in_=t['w_out_T'][g * 128:(g + 1) * 128,
                                         mb * 128:(mb + 1) * 128])
                    ost = P.tile([128, L], BF16, tag='ost', bufs=3,
                                 name=f'os{mb}{g}')
                    if g == 0:
                        po = PS.tile([128, L], F32, tag='py', bufs=1,
                                     name=f'po{mb}{g}')
                        for c in range(TC4):
                            nc.tensor.matmul(
                                out=po[:, c * CH:(c + 1) * CH], lhsT=wo[:],
                                rhs=yg[:, c * CH:(c + 1) * CH],
                                start=True, stop=True)
                        nc.scalar.activation(out=ost[:], in_=po[:],
                                             func=AF.Copy)
                    else:
                        for c in range(TC4):
                            po = PS.tile([128, CH], F32, tag='pa', bufs=2,
                                         name=f'po{mb}{g}{c}')
                            nc.tensor.matmul(
                                out=po[:], lhsT=wo[:],
                                rhs=yg[:, c * CH:(c + 1) * CH],
                                start=True, stop=True)
                            if (mb * TC4 + c) % 2 == 0:
                                nc.scalar.activation(
                                    out=ost[:, c * CH:(c + 1) * CH],
                                    in_=po[:], func=AF.Copy)
                            else:
                                nc.vector.tensor_copy(
                                    ost[:, c * CH:(c + 1) * CH], po[:])
                    nc.sync.dma_start(
                        out=t[f'rs_in{g}'][mb * 128:(mb + 1) * 128, :],
                        in_=ost[:])
                # RS for this g; g=0's runs hidden under g=1's scan loop
                nc.gpsimd.collective_compute(
                    'ReduceScatter', ALU.add,
                    replica_groups=[list(range(NCORES))],
                    ins=[t[f'rs_in{g}'][:]], outs=[t[f'rs_out{g}'][:]])


            ro0 = P.tile([128, L], BF16, tag='yg', bufs=2, name='ro0')
            nc.sync.dma_start(out=ro0[:], in_=t['rs_out0'][:])
            ro1 = P.tile([128, L], BF16, tag='ost', bufs=3, name='ro1')
            nc.sync.dma_start(out=ro1[:], in_=t['rs_out1'][:])
            outf = P.tile([128, L], BF16, tag='ost', bufs=3, name='outf')
            nc.vector.tensor_tensor(out=outf[:], in0=ro0[:], in1=ro1[:],
                                    op=ALU.add)
            nc.sync.dma_start(out=t['out_mT'][:], in_=outf[:])


def _build():
    nc = bacc.Bacc(None, target_bir_lowering=False)

    def inp(name, shape, dt=F32):
        return nc.declare_dram_parameter(name, shape, dt, isOutput=False)

    t = {
        'h_T': inp('h_T', [DM, L], BF16),
        'w_in_T': inp('w_in_T', [DM, 2 * DLOC], BF16),
        'w_conv': inp('w_conv', [(2 * NG * KC + 2 * NG) * 128, 128], BF16),
        'wx_T': inp('wx_T', [DLOC, 2 * 96], BF16),
        'wdt_T': inp('wdt_T', [R, 2 * DLOC], BF16),
        'w_out_T': inp('w_out_T', [DLOC, DM], BF16),
        'bdt': inp('bdt', [DLOC, 2]),
        'conv_b': inp('conv_b', [DLOC, 2]),
        'd_both': inp('d_both', [DLOC, 2]),
        'A_cols': inp('A_cols', [DLOC, 2 * N]),
        'ident': inp('ident', [128, 128], BF16),
        'out_mT': nc.declare_dram_parameter('out_mT', [128, L], BF16,
                                            isOutput=True),
        'rs_in0': nc.dram_tensor('rs_in0', [DM, L], BF16),
        'rs_in1': nc.dram_tensor('rs_in1', [DM, L], BF16),
        'rs_out0': nc.dram_tensor('rs_out0', [128, L], BF16),
        'rs_out1': nc.dram_tensor('rs_out1', [128, L], BF16),
    }
    t['warm_in'] = nc.dram_tensor('warm_in', [8, 16], BF16)
    t['warm_out'] = nc.dram_tensor('warm_out', [8, 16], BF16,
                                   addr_space='Shared')
    for d in range(2):
        t[f'ar_in{d}'] = nc.dram_tensor(f'ar_in{d}', [96, L], BF16)
        t[f'ar_out{d}'] = nc.dram_tensor(f'ar_out{d}', [96, L], BF16,
                                         addr_space='Shared')
    _emit(nc, t)
    nc.compile()
    return nc


def prepare_in_maps(inputs):
    f32 = np.float32
    bf16 = ml_dtypes.bfloat16
    h = np.asarray(inputs['hidden_states'], f32)[0]        # [L, DM]
    h_T = np.ascontiguousarray(h.T).astype(bf16)
    W_in = np.asarray(inputs['W_in'], f32)
    W_out = np.asarray(inputs['W_out'], f32)

    maps = []
    for c in range(NCORES):
        sl = slice(c * DLOC, (c + 1) * DLOC)
        cw_f = np.asarray(inputs['conv_w_fwd'], f32)[sl]   # [DLOC, 4]
        cw_r = np.asarray(inputs['conv_w_rev'], f32)[sl]
        w_conv = np.zeros(((2 * NG * KC + 2 * NG) * 128, 128), f32)
        for d in range(2):
            for g in range(NG):
                for k in range(KC):
                    b = (d * NG + g) * KC + k
                    ch = slice(g * 128, (g + 1) * 128)
                    tap = cw_f[ch, k] if d == 0 else cw_r[ch, KC - 1 - k]
                    w_conv[b * 128:(b + 1) * 128, :] = np.diag(tap)
        dp_f = np.asarray(inputs['D_fwd'], f32)[sl]
        dp_r = np.asarray(inputs['D_rev'], f32)[sl]
        for d in range(2):
            for g in range(NG):
                b = 2 * NG * KC + d * NG + g
                ch = slice(g * 128, (g + 1) * 128)
                w_conv[b * 128:(b + 1) * 128, :] = np.diag(
                    (dp_f if d == 0 else dp_r)[ch])
        a_cols = np.concatenate(
            [-np.exp(np.asarray(inputs['A_log_fwd'], f32)[sl]),
             -np.exp(np.asarray(inputs['A_log_rev'], f32)[sl])], axis=1)
        w_in_T = np.concatenate(
            [W_in[sl].T, W_in[DI + c * DLOC:DI + (c + 1) * DLOC].T], axis=1)
        m = {
            'h_T': h_T,
            'w_in_T': np.ascontiguousarray(w_in_T).astype(bf16),
            'w_conv': w_conv.astype(bf16),
            'wx_T': np.ascontiguousarray(np.concatenate(
                [np.asarray(inputs['Wx_fwd'], f32)[:, sl].T,
                 np.asarray(inputs['Wx_rev'], f32)[:, sl].T],
                axis=1)).astype(bf16),
            'wdt_T': np.ascontiguousarray(np.concatenate(
                [np.asarray(inputs['Wdt_fwd'], f32)[sl].T,
                 np.asarray(inputs['Wdt_rev'], f32)[sl].T],
                axis=1)).astype(bf16),
            'w_out_T': np.ascontiguousarray(W_out[:, sl].T).astype(bf16),
            'bdt': np.ascontiguousarray(np.stack(
                [np.asarray(inputs['bdt_fwd'], f32)[sl],
                 np.asarray(inputs['bdt_rev'], f32)[sl]], axis=1)),
            'conv_b': np.ascontiguousarray(np.stack(
                [np.asarray(inputs['conv_b_fwd'], f32)[sl],
                 np.asarray(inputs['conv_b_rev'], f32)[sl]], axis=1)),
            'd_both': np.ascontiguousarray(np.stack(
                [np.asarray(inputs['D_fwd'], f32)[sl],
                 np.asarray(inputs['D_rev'], f32)[sl]], axis=1)),
            'A_cols': np.ascontiguousarray(a_cols),
            'ident': np.eye(128, dtype=f32).astype(bf16),
        }
        maps.append(m)
    return maps


def get_nc():
    if 'nc' not in _CACHE:
        _CACHE['nc'] = _build()
    return _CACHE['nc']


def run(inputs, **kw):
    nc = get_nc()
    maps = prepare_in_maps(inputs)
    res = run_bass_kernel_spmd(nc, maps, list(range(NCORES)), **kw)
    out_T = np.concatenate(
        [np.asarray(res.results[c]['out_mT'], np.float32)
         for c in range(NCORES)], axis=0)                 # [DM, L]
    out = np.ascontiguousarray(out_T.T)[None]             # [1, L, DM]
    return out.astype(np.float32), res


def kernel(**inputs):
    out, _ = run(inputs)
    return out
